# revision 1
# baseline (speedup 1.0000x reference)
"""Trainium2 Bass kernel for a 4-layer compressed model:

    for l in range(4):  x = x @ (base[l] + bitdelta[l] * mask[l])

x: [16, 4096] f32, base/mask: [4, 4096, 4096] f32, bitdelta: [4] f32.

Sharding (8 cores, tensor parallel on weight columns):
  core c owns columns [c*512, (c+1)*512) of every layer's weight.

Key ideas:
  * Low-precision streaming: base is cast to bf16 on the host (quant
    noise ~0.2% of base, itself ~25x smaller than bd*mask); mask is
    +/-1 exactly, which fp8e4m3 represents exactly. Activations ride
    in bf16. HBM traffic per core: 64 MiB (f32) -> 24 MiB. bitdelta
    values are baked into the program as immediates (compiled after
    inputs are known; cache keyed on them).
  * The dense weight W = base + bd*mask is reconstructed ON-CHIP by the
    DVE (one scalar_tensor_tensor per 1 MiB chunk) as the chunks land.
    The combine depends only on the weight stream — never on gathered
    activations — so it always runs ahead, and the PE does a single
    matmul per k-tile (32/layer).
  * Queue discipline: ALL weight DMAs are issued up front on the sync
    queue (nothing gather-dependent ever blocks the stream); activation
    staging (y^T to DRAM) and x^T reloads ride the scalar queue; the
    collectives ride gpsimd; combines ride the DVE. Three layers of
    combined weights buffer in SBUF, so the stream runs through every
    gather wait.
  * The per-layer accumulation is split into two 256-column halves:
    the left half's PSUM drain + PE transposes overlap the right
    half's matmuls, shortening the post-matmul tail before each
    AllGather (and letting the final output DMA start early).
  * Between layers the [16,512] local result is PE-transposed to
    [512,16] bf16 and AllGather'd on the partition axis into the next
    layer's x^T — exactly the lhsT layout the next matmuls need.
  * PE warmers: a chain of throwaway matmuls anchored on yt_sb (ready
    exactly when the gather is triggered) runs through each gather
    window, holding the HAM clock gate at 2.4 GHz. Cold restarts both
    slow the next layer 2x and skew cores apart, and a collective
    completes only when the slowest rank arrives.

Memory-bound: each core streams 24 MiB of weights; roofline ~70 us.
"""

import numpy as np

import concourse.bass as bass
import concourse.mybir as mybir
import concourse.tile as tile
from concourse import bacc
from concourse.bass_utils import run_bass_kernel_spmd
from concourse.masks import make_identity

L = 4
D = 4096
B = 16
NCORES = 8
C = D // NCORES          # 512 columns per core
CH = C // 2              # 256-column accumulator halves
KT = D // 128            # 32 contraction tiles of 128
GKB = 8                  # k-tiles per base DMA (1 MiB bf16 transfers)
NGB = KT // GKB          # 4 base DMAs per layer
GKM = 16                 # k-tiles per mask DMA (1 MiB fp8 transfers)
NGM = KT // GKM          # 2 mask DMAs per layer
XCH = 4                  # x^T load chunks per layer
KXC = KT // XCH          # k-tiles per x chunk
CT = C // 128            # 4 transpose chunks
WBUFS_B = 6              # raw base tiles in flight
WBUFS_M = 3              # raw mask tiles in flight
WBUFS_C = 12             # combined tiles in flight (12 MiB, 3 layers)
NWARM = 32               # PE-warmer matmuls per gather window

F32 = mybir.dt.float32
BF16 = mybir.dt.bfloat16
FP8 = mybir.dt.float8e4
ALU = mybir.AluOpType

_cache = {}


def build(bd_vals):
    nc = bacc.Bacc(
        "TRN2",
        target_bir_lowering=False,
        debug=False,
        num_devices=NCORES,
    )

    # x^T in natural [4096, 16] order; row d = p*KT + k maps to SBUF
    # partition p, matmul index k — so the load is partition-contiguous.
    xT0 = nc.dram_tensor("xT0", [D, B], BF16, kind="ExternalInput")
    # weight shards, pre-permuted on host: [l, g, p, j*C+c] = W_l[p*KT+g*GK+j,
    # c]; each block is 1 MiB contiguous.
    base_sh = nc.dram_tensor("base_sh", [L, NGB, 128, GKB * C], BF16,
                             kind="ExternalInput")
    mask_sh = nc.dram_tensor("mask_sh", [L, NGM, 128, GKM * C], FP8,
                             kind="ExternalInput")
    out = nc.dram_tensor("out", [B, C], F32, kind="ExternalOutput")

    rg = [list(range(NCORES))]

    def load_xt_chunks(xpool, src):
        """Load x^T [D, B] into 4 SBUF chunk tiles of 8 k-tiles each."""
        chunks = []
        for xc in range(XCH):
            xt = xpool.tile([128, KXC * B], BF16, tag=f"xt{xc}")
            nc.scalar.dma_start(
                xt[:, :].rearrange("p (k b) -> p k b", k=KXC),
                src[:, :].rearrange("(p k) b -> p k b", p=128)
                [:, xc * KXC:(xc + 1) * KXC],
            )
            chunks.append(xt)
        return chunks

    with tile.TileContext(nc) as tc:
        with (
            tc.tile_pool(name="wb", bufs=WBUFS_B) as bpool,
            tc.tile_pool(name="wm", bufs=WBUFS_M) as mpool,
            tc.tile_pool(name="wc", bufs=WBUFS_C) as wcpool,
            tc.tile_pool(name="xp", bufs=2) as xpool,
            tc.tile_pool(name="sp", bufs=2) as spool,
            tc.tile_pool(name="const", bufs=1) as cpool,
            tc.tile_pool(name="acc", bufs=2, space="PSUM") as psum,
            tc.tile_pool(name="tp", bufs=2, space="PSUM") as tpsum,
            tc.tile_pool(name="warm", bufs=1, space="PSUM") as wpsum,
            tc.tile_pool(name="dram", bufs=2, space="DRAM") as dram,
        ):
            ident = cpool.tile([B, B], F32, tag="ident")
            make_identity(nc, ident[:, :])

            # Issue the whole weight stream + on-chip combines up front.
            # The sync queue carries only weight DMAs, so it never blocks
            # on activations; DVE carries only the combines.
            wcs = []
            for l in range(L):
                bd = float(bd_vals[l])
                wms = []
                for g in range(NGM):
                    wm = mpool.tile([128, GKM * C], FP8, tag="wm")
                    nc.sync.dma_start(wm[:, :], mask_sh[l, g])
                    wms.append(wm)
                wcs.append([])
                for g in range(NGB):
                    wb = bpool.tile([128, GKB * C], BF16, tag="wb")
                    nc.sync.dma_start(wb[:, :], base_sh[l, g])
                    wc = wcpool.tile([128, GKB * C], BF16, tag="wc")
                    half = (g % 2) * (GKB * C)
                    nc.vector.scalar_tensor_tensor(
                        out=wc[:, :],
                        in0=wms[g // 2][:, half:half + GKB * C],
                        scalar=bd,
                        in1=wb[:, :],
                        op0=ALU.mult,
                        op1=ALU.add,
                    )
                    wcs[l].append(wc)

            xts = load_xt_chunks(xpool, xT0)

            for l in range(L):
                last = l == L - 1
                y_sb = spool.tile([B, C], F32, tag="y")
                yt_sb = None if last else spool.tile([128, CT * B], BF16,
                                                     tag="yt")
                # Two 256-column halves: the left half's drain/transposes
                # overlap the right half's matmuls.
                for h in range(2):
                    acc = psum.tile([B, CH], F32, tag=f"acc{h}")
                    for k in range(KT):
                        xc, kk = k // KXC, k % KXC
                        j = k % GKB
                        nc.tensor.matmul(
                            acc[:, :],
                            xts[xc][:, kk * B:(kk + 1) * B],
                            wcs[l][k // GKB][:, j * C + h * CH:
                                             j * C + (h + 1) * CH],
                            start=(k == 0),
                            stop=(k == KT - 1),
                        )
                    nc.scalar.copy(y_sb[:, h * CH:(h + 1) * CH], acc[:, :])
                    if last:
                        nc.scalar.dma_start(out[:, h * CH:(h + 1) * CH],
                                            y_sb[:, h * CH:(h + 1) * CH])
                    else:
                        for t in range(2):
                            cc = h * 2 + t
                            pt = tpsum.tile([128, B], F32, tag="pt")
                            nc.tensor.transpose(
                                pt[:, :],
                                y_sb[:, cc * 128:(cc + 1) * 128],
                                ident[:, :],
                            )
                            nc.scalar.copy(
                                yt_sb[:, cc * B:(cc + 1) * B], pt[:, :]
                            )

                if not last:
                    ytb = dram.tile([C, B], BF16, tag="ytb")
                    nc.scalar.dma_start(
                        ytb[:, :].rearrange("(cc p) b -> p cc b", p=128),
                        yt_sb[:, :].rearrange("p (cc b) -> p cc b", cc=CT),
                    )
                    xt_full = dram.tile([D, B], BF16, tag="xtf",
                                        addr_space="Shared")
                    nc.gpsimd.collective_compute(
                        "AllGather",
                        ALU.bypass,
                        replica_groups=rg,
                        ins=[ytb.opt()],
                        outs=[xt_full.opt()],
                    )

                    # PE warmers: anchored on yt_sb (ready right at gather
                    # trigger), they run back-to-back through the gather
                    # window on next-layer weights already in SBUF.
                    warm_ps = wpsum.tile([B, C], F32, tag="warm")
                    wsrc = wcs[l + 1][0]
                    for i in range(NWARM):
                        nc.tensor.matmul(
                            warm_ps[:, :],
                            yt_sb[:, :B],
                            wsrc[:, :C],
                            start=(i == 0),
                            stop=(i == NWARM - 1),
                        )

                    xts = load_xt_chunks(xpool, xt_full)

    nc.compile()
    return nc


def _get_nc(bd_vals):
    key = tuple(float(v) for v in bd_vals)
    if _cache.get("key") != key:
        _cache["nc"] = build(bd_vals)
        _cache["key"] = key
    return _cache["nc"]


def _shard_weight(w, gk):
    """[L, D, C] column shard -> [L, KT//gk, 128, gk*C] with
    out[l, g, p, j*C + c] = w[l, p*KT + g*gk + j, c]."""
    ng = KT // gk
    w = w.reshape(L, 128, ng, gk, C)
    w = w.transpose(0, 2, 1, 3, 4)            # [L, ng, 128, gk, C]
    return np.ascontiguousarray(w.reshape(L, ng, 128, gk * C))


def _make_in_maps(x, base, mask, bitdelta):
    import ml_dtypes

    x = np.ascontiguousarray(x, dtype=np.float32)
    base = np.asarray(base, dtype=np.float32)
    mask = np.asarray(mask, dtype=np.float32)

    xT = np.ascontiguousarray(x.T).astype(ml_dtypes.bfloat16)    # [D, B]

    base16 = base.astype(ml_dtypes.bfloat16)
    mask8 = mask.astype(ml_dtypes.float8_e4m3)

    in_maps = []
    for c in range(NCORES):
        sl = slice(c * C, (c + 1) * C)
        in_maps.append({
            "xT0": xT,
            "base_sh": _shard_weight(base16[:, :, sl], GKB),
            "mask_sh": _shard_weight(mask8[:, :, sl], GKM),
        })
    return in_maps


def _run(x, base, mask, bitdelta, trace=False):
    nc = _get_nc(np.asarray(bitdelta, dtype=np.float32))
    in_maps = _make_in_maps(x, base, mask, bitdelta)
    res = run_bass_kernel_spmd(
        nc, in_maps, core_ids=list(range(NCORES)), trace=trace
    )
    y = np.concatenate([res.results[c]["out"] for c in range(NCORES)], axis=1)
    return y, res


def kernel(x, base, mask, bitdelta):
    y, _ = _run(x, base, mask, bitdelta)
    return y



# revision 9
# speedup vs baseline: 1.0342x; 1.0342x over previous
"""Trainium2 Bass kernel for a 4-layer compressed model:

    for l in range(4):  x = x @ (base[l] + bitdelta[l] * mask[l])

x: [16, 4096] f32, base/mask: [4, 4096, 4096] f32, bitdelta: [4] f32.

Sharding (8 cores, tensor parallel on weight columns):
  core c owns columns [c*512, (c+1)*512) of every layer's weight.

Key ideas vs the previous version (160 us):
  * NO on-chip weight reconstruction. The dense-combine DVE pass
    (bd*mask + base, 70 us of serial vector work at 1x mode) is gone:
      x @ (base + bd*mask) = (bd*x) @ mask  +  x @ (64*base) / 64
    Both weight streams ride fp8e4 (mask is +/-1, EXACT in fp8;
    64*base spans +-7 with ~2.4% relative error on a term that is only
    ~2.5% of the output). HBM traffic per core: 24 MiB -> 16 MiB.
  * The two matmul streams run CONCURRENTLY on the PE array via column
    tiling: bd*x stationary in array cols 0-15 (tile_position (0,0)),
    x stationary in cols 32-47 ((0,32)). Both accumulate in the same
    PSUM bank at partitions 0-15 / 32-47. Mixed dtype (bf16 stationary,
    fp8 moving) works on HW. ~32 pair-issues of N=512 per layer.
  * The mask-acc + base-acc/64 recombine AND the [16,512] -> [512,16]
    transpose for the next layer's lhsT happen in one PE pass: per
    128-column chunk, two accumulating matmuls (stack chunk stationary,
    selector moving: cols 0-15 I for the mask stack, cols 16-31 I/64
    for the base stack) produce yT = acc_mask.T + acc_base.T/64 in
    PSUM. Both stacks sit at partitions 0-15 (the base acc is drained
    PSUM[32:48] -> SBUF[0:16] by the ACT engine, which can shift
    partitions); keeping every sel-matmul in PE row group 0 matters --
    alternating stationary row groups 0/32 inside the accumulation
    stream is a hard device crash (NRT_EXEC_UNIT_UNRECOVERABLE).
  * Weights stream as 16 x 1 MiB DMAs issued up front on the sync
    HWDGE ring; all 16 chunk tiles stay resident in SBUF (no buffer
    recycling -> the stream never stalls on compute). Activation
    staging / reloads ride the scalar (ACT) ring; collectives ride
    gpsimd; 3 AllGathers of yT [512,16] bf16 between layers.
  * Short PE warmer chains (anchored on each layer's yT, running on
    the next layer's already-resident weights) hold the HAM clock gate
    at 2.4 GHz through each gather window.

Output is written transposed ([512,16] f32 per core); the host
concatenates and transposes back.
"""

import numpy as np

import concourse.bass as bass
import concourse.mybir as mybir
import concourse.tile as tile
from concourse import bacc
from concourse.bass_utils import run_bass_kernel_spmd

L = 4
D = 4096
B = 16
NCORES = 8
C = D // NCORES          # 512 columns per core
KT = D // 128            # 32 contraction tiles of 128
GK = 8                   # k-tiles per DMA chunk
NCH = KT // GK           # 4 chunks per layer (1 MiB each: mask+base)
CT = C // 128            # 4 transpose chunks
SCALE = 64.0             # base pre-scale (fp8 denormal avoidance)
NWARM = 40               # PE-warmer matmuls per gather window

F32 = mybir.dt.float32
BF16 = mybir.dt.bfloat16
FP8 = mybir.dt.float8e4
ALU = mybir.AluOpType

_cache = {}


def build(bd_vals):
    nc = bacc.Bacc(
        "TRN2",
        target_bir_lowering=False,
        debug=False,
        num_devices=NCORES,
    )

    # x^T in natural [4096, 16] order; row d = p*KT + k maps to SBUF
    # partition p, matmul k-tile k -- the load is partition-contiguous.
    xT0 = nc.dram_tensor("xT0", [D, B], BF16, kind="ExternalInput")
    # selector for the recombining transposes: cols 0-15 I, cols 16-31 I/64
    sel = nc.dram_tensor("sel", [B, 2 * B], BF16, kind="ExternalInput")
    # weight chunks: [l, g, p, :GK*C] = mask[l, p*KT+g*GK+j, c] (fp8, +-1)
    #               [l, g, p, GK*C:] = 64*base[l, p*KT+g*GK+j, c] (fp8)
    w8 = nc.dram_tensor("w8", [L, NCH, 128, 2 * GK * C], FP8,
                        kind="ExternalInput")
    outT = nc.dram_tensor("outT", [C, B], F32, kind="ExternalOutput")

    rg = [list(range(NCORES))]

    with tile.TileContext(nc) as tc:
        with (
            tc.tile_pool(name="w", bufs=L * NCH) as wpool,
            tc.tile_pool(name="xp", bufs=2) as xpool,
            tc.tile_pool(name="sp", bufs=2) as spool,
            tc.tile_pool(name="const", bufs=1) as cpool,
            tc.tile_pool(name="acc", bufs=2, space="PSUM") as psum,
            tc.tile_pool(name="tp", bufs=2, space="PSUM") as tpsum,
            tc.tile_pool(name="warm", bufs=1, space="PSUM") as wpsum,
            tc.tile_pool(name="dram", bufs=2, space="DRAM") as dram,
        ):
            # whole weight stream up front on the sync ring; every chunk
            # tile stays resident (16 x 8 KiB/partition = 128 KiB).
            wt = []
            for l in range(L):
                for g in range(NCH):
                    t = wpool.tile([128, 2 * GK * C], FP8, tag="w")
                    nc.sync.dma_start(t[:, :], w8[l, g])
                    wt.append(t)

            sel_sb = cpool.tile([B, 2 * B], BF16, tag="sel")
            nc.scalar.dma_start(sel_sb[:, :], sel[:, :])

            xt = xpool.tile([128, KT * B], BF16, tag="xt")
            nc.scalar.dma_start(
                xt[:, :].rearrange("p (k b) -> p k b", k=KT),
                xT0[:, :].rearrange("(p k) b -> p k b", p=128),
            )

            for l in range(L):
                last = l == L - 1
                bd = float(bd_vals[l])

                xm = xpool.tile([128, KT * B], BF16, tag="xm")
                nc.vector.tensor_scalar_mul(xm[:, :], xt[:, :], bd)

                ps = psum.tile([48, C], F32, tag="ps")
                for k in range(KT):
                    g, j = k // GK, k % GK
                    wti = wt[l * NCH + g]
                    nc.tensor.matmul(
                        ps[0:16, :],
                        xm[:, k * B:(k + 1) * B],
                        wti[:, j * C:(j + 1) * C],
                        start=(k == 0), stop=(k == KT - 1),
                        tile_position=(0, 0), skip_group_check=True,
                    )
                    nc.tensor.matmul(
                        ps[32:48, :],
                        xt[:, k * B:(k + 1) * B],
                        wti[:, GK * C + j * C:GK * C + (j + 1) * C],
                        start=(k == 0), stop=(k == KT - 1),
                        tile_position=(0, 32), skip_group_check=True,
                    )

                # drain both accumulators to partition-0 stacks (ACT can
                # shift partitions: PSUM[32:48] -> SBUF[0:16])
                stka = spool.tile([B, C], BF16, tag="stka")
                stkb = spool.tile([B, C], BF16, tag="stkb")
                nc.scalar.copy(stka[:, :], ps[0:16, :])
                nc.scalar.copy(stkb[:, :], ps[32:48, :])

                # recombine + transpose in one PE pass (regular matmuls,
                # stack chunk stationary, selector moving, all row grp 0):
                # yT_chunk = mask_acc.T @ I + base_acc.T @ (I/64)
                yt_ps = tpsum.tile([128, CT * B], F32, tag="ytps")
                for cc in range(CT):
                    nc.tensor.matmul(
                        yt_ps[:, cc * B:(cc + 1) * B],
                        stka[:, cc * 128:(cc + 1) * 128],
                        sel_sb[:, 0:B],
                        start=True, stop=False,
                        skip_group_check=True,
                    )
                    nc.tensor.matmul(
                        yt_ps[:, cc * B:(cc + 1) * B],
                        stkb[:, cc * 128:(cc + 1) * 128],
                        sel_sb[:, B:2 * B],
                        start=False, stop=True,
                        skip_group_check=True,
                    )

                if last:
                    ytf = spool.tile([128, CT * B], F32, tag="ytf")
                    nc.scalar.copy(ytf[:, :], yt_ps[:, :])
                    nc.scalar.dma_start(
                        outT[:, :].rearrange("(cc p) b -> p cc b", p=128),
                        ytf[:, :].rearrange("p (cc b) -> p cc b", cc=CT),
                    )
                else:
                    yt_sb = spool.tile([128, CT * B], BF16, tag="ytsb")
                    nc.scalar.copy(yt_sb[:, :], yt_ps[:, :])

                    ytb = dram.tile([C, B], BF16, tag="ytb")
                    nc.scalar.dma_start(
                        ytb[:, :].rearrange("(cc p) b -> p cc b", p=128),
                        yt_sb[:, :].rearrange("p (cc b) -> p cc b", cc=CT),
                    )
                    xt_full = dram.tile([D, B], BF16, tag="xtf",
                                        addr_space="Shared")
                    nc.gpsimd.collective_compute(
                        "AllGather",
                        ALU.bypass,
                        replica_groups=rg,
                        ins=[ytb.opt()],
                        outs=[xt_full.opt()],
                    )

                    # PE warmers through the gather window: anchored on
                    # yt_sb, next layer's (resident) weights as moving.
                    warm = wpsum.tile([B, 128], F32, tag="warm")
                    wsrc = wt[(l + 1) * NCH]
                    for i in range(NWARM):
                        nc.tensor.matmul(
                            warm[:, :],
                            yt_sb[:, :B],
                            wsrc[:, :128],
                            start=(i == 0), stop=(i == NWARM - 1),
                            skip_group_check=True,
                        )

                    xt = xpool.tile([128, KT * B], BF16, tag="xt")
                    nc.scalar.dma_start(
                        xt[:, :].rearrange("p (k b) -> p k b", k=KT),
                        xt_full[:, :].rearrange("(p k) b -> p k b", p=128),
                    )

    nc.compile()
    return nc


def _get_nc(bd_vals):
    key = tuple(float(v) for v in bd_vals)
    if _cache.get("key") != key:
        _cache["nc"] = build(bd_vals)
        _cache["key"] = key
    return _cache["nc"]


def _make_in_maps(x, base, mask, bitdelta):
    import ml_dtypes

    x = np.ascontiguousarray(np.asarray(x, dtype=np.float32))
    base = np.asarray(base, dtype=np.float32)
    mask = np.asarray(mask, dtype=np.float32)

    xT = np.ascontiguousarray(x.T).astype(ml_dtypes.bfloat16)    # [D, B]

    sel = np.zeros((B, 2 * B), dtype=np.float32)
    sel[:, 0:B] = np.eye(B, dtype=np.float32)
    sel[:, B:2 * B] = np.eye(B, dtype=np.float32) / SCALE
    sel = sel.astype(ml_dtypes.bfloat16)

    mask8 = mask.astype(ml_dtypes.float8_e4m3)
    base8 = (base * SCALE).astype(ml_dtypes.float8_e4m3)

    in_maps = []
    for c in range(NCORES):
        sl = slice(c * C, (c + 1) * C)
        m = mask8[:, :, sl].reshape(L, 128, KT, C)   # row d = p*KT + k
        b = base8[:, :, sl].reshape(L, 128, KT, C)
        w = np.empty((L, NCH, 128, 2 * GK * C), dtype=ml_dtypes.float8_e4m3)
        for g in range(NCH):
            gs = slice(g * GK, (g + 1) * GK)
            w[:, g, :, :GK * C] = m[:, :, gs, :].reshape(L, 128, GK * C)
            w[:, g, :, GK * C:] = b[:, :, gs, :].reshape(L, 128, GK * C)
        in_maps.append({"xT0": xT, "sel": sel, "w8": w})
    return in_maps


def _assemble(outTs):
    outT = np.concatenate(outTs, axis=0)             # [D, B]
    return np.ascontiguousarray(outT.T.astype(np.float32))


def _run(x, base, mask, bitdelta, trace=False):
    nc = _get_nc(np.asarray(bitdelta, dtype=np.float32))
    in_maps = _make_in_maps(x, base, mask, bitdelta)
    res = run_bass_kernel_spmd(
        nc, in_maps, core_ids=list(range(NCORES)), trace=trace
    )
    y = _assemble([res.results[c]["outT"] for c in range(NCORES)])
    return y, res


def kernel(x, base, mask, bitdelta):
    y, _ = _run(x, base, mask, bitdelta)
    return y
